# revision 4
# baseline (speedup 1.0000x reference)
"""Trainium2 Bass kernel for the CachedMPS classifier (nn_CachedMPS_68212670595935).

Matrix Product State classifier:
  feats = (cos(pi/2 x), sin(pi/2 x))                        [B, N, 2]
  M0 = feats[:,0,:] @ core0                                 [B, D]
  scan over 782 mid sites: M <- l2norm(M @ (c0*A0 + c1*A1))
  logits = einsum(M, feats[:,-1,:], core_last)              [B, C]

Key reformulation: the per-step L2 normalization is a pure per-row scale and the
step map is linear in M, so every intermediate normalization cancels in the final
normalization (eps=1e-8 perturbs at ~1e-8 relative - negligible). The device scan
runs UN-normalized with a constant alpha folded into the mid cores for fp32 range
control, and normalizes once at the end. Verified in fp64/fp32 numpy: 7e-6 rel err.

Device layout (per core, batch shard Bs=1024, data-parallel over 8 cores):
  state V [128, Bs] = [c0 * M ; c1 * M] (transposed, aug-doubled), fp32r in SBUF.
  step n: P2 = Wdoub_n^T @ V   (PE, fp32r, Wdoub = [alpha*Aaug | alpha*Aaug], 2 MMs of N=512)
          Crep = E^T @ stage_n (PE, replicates the site's (c0,c1) rows to 128 partitions)
          CrepSB <- Crep        (ACT copy PSUM->SBUF)
          V' = P2 * CrepSB      (DVE tensor_tensor, the only per-step DVE op)
  Features cos/sin computed on device with ACT Sin; per-step 8KB SBUF->SBUF DMA
  stages the site's two feature rows for the replicate matmul.
"""

import math
import numpy as np
from contextlib import ExitStack

import concourse.bass as bass
import concourse.tile as tile
from concourse import bacc, mybir
from concourse.bass_utils import run_bass_kernel_spmd

F32 = mybir.dt.float32
F32R = mybir.dt.float32r
AF = mybir.ActivationFunctionType

D = 64
C = 10
N_CORES = 8
HALF_PI = float(np.pi / 2.0)
ALPHA = float(2.0 ** (-1.0 / 3.0))


def build_nc(n_sites: int, Bs: int, chunk: int = 512):
    """Build the per-core Bass program. Returns (nc, names) with names of dram tensors."""
    n_mid = n_sites - 2
    n_ftiles = (n_sites + 127) // 128
    n_chunks = Bs // chunk
    assert Bs % chunk == 0 and chunk >= 256

    nc = bacc.Bacc("TRN2", target_bir_lowering=False, debug=False)

    xT = nc.dram_tensor("xT", [n_sites, Bs], F32, kind="ExternalInput")
    Ad = nc.dram_tensor("Ad", [n_mid, 128, 128], F32R, kind="ExternalInput")
    W0 = nc.dram_tensor("W0", [2, 128], F32R, kind="ExternalInput")
    WL = nc.dram_tensor("WL", [128, C], F32R, kind="ExternalInput")
    Esel = nc.dram_tensor("Esel", [2, 128], F32R, kind="ExternalInput")
    ones64 = nc.dram_tensor("ones64", [D, 1], F32R, kind="ExternalInput")
    ones10 = nc.dram_tensor("ones10", [1, C], F32R, kind="ExternalInput")
    out = nc.dram_tensor("out", [C, Bs], F32, kind="ExternalOutput")

    with tile.TileContext(nc) as tc:
        with ExitStack() as ctx:
            const = ctx.enter_context(tc.tile_pool(name="const", bufs=1))
            vpool = ctx.enter_context(tc.tile_pool(name="vpool", bufs=2))
            crep_sb = ctx.enter_context(tc.tile_pool(name="crep_sb", bufs=2))
            wpool = ctx.enter_context(tc.tile_pool(name="wpool", bufs=4))
            stpool = ctx.enter_context(tc.tile_pool(name="stpool", bufs=4))
            endp = ctx.enter_context(tc.tile_pool(name="endp", bufs=1))
            pp = ctx.enter_context(tc.tile_pool(name="pp", bufs=2, space="PSUM"))
            cp = ctx.enter_context(tc.tile_pool(name="cp", bufs=2, space="PSUM"))

            # ---- constants / weights resident in SBUF
            e_sb = const.tile([2, 128], F32R)
            nc.sync.dma_start(e_sb[:], Esel.ap())
            w0_sb = const.tile([2, 128], F32R)
            nc.sync.dma_start(w0_sb[:], W0.ap())
            wl_sb = const.tile([128, C], F32R)
            nc.sync.dma_start(wl_sb[:], WL.ap())
            o64_sb = const.tile([D, 1], F32R)
            nc.sync.dma_start(o64_sb[:], ones64.ap())
            o10_sb = const.tile([1, C], F32R)
            nc.sync.dma_start(o10_sb[:], ones10.ap())

            # ---- feature build: fsb[p, f, t*Bs + b] = cos/sin(pi/2 * x[site=t*128+p, b])
            xsb = const.tile([128, n_ftiles, Bs], F32)
            if n_sites % 128 != 0:
                nc.vector.memset(xsb[:], 0.0)
            for t in range(n_ftiles):
                rows = min(128, n_sites - t * 128)
                nc.sync.dma_start(xsb[:rows, t, :], xT.ap()[t * 128:t * 128 + rows, :])
            fsb = const.tile([128, 2, n_ftiles, Bs], F32R)
            # cos = Sin(pi/2 * x + pi/2), sin = Sin(pi/2 * x)
            b_half = const.tile([128, 1], F32)
            nc.vector.memset(b_half[:], HALF_PI)
            b_zero = const.tile([128, 1], F32)
            nc.vector.memset(b_zero[:], 0.0)
            nc.scalar.activation(fsb[:, 0], xsb[:], AF.Sin, bias=b_half[:], scale=HALF_PI)
            nc.scalar.activation(fsb[:, 1], xsb[:], AF.Sin, bias=b_zero[:], scale=HALF_PI)

            def stage_site(s):
                st = stpool.tile([2, Bs], F32R, tag="stage")
                p, t = s % 128, s // 128
                nc.sync.dma_start(st[:], fsb[p:p + 1, :, t, :])
                return st

            def mm_pair(ps, lhsT, rhs_tile):
                for c in range(n_chunks):
                    nc.tensor.matmul(ps[:, c * chunk:(c + 1) * chunk], lhsT,
                                     rhs_tile[:, c * chunk:(c + 1) * chunk],
                                     start=True, stop=True)

            # ---- site 0: P2_0 = [W0 | W0]^T @ stage0  -> [128, Bs] (both halves = M0)
            st0 = stage_site(0)
            p2 = pp.tile([128, Bs], F32, tag="p2")
            mm_pair(p2, w0_sb[:], st0[:])

            # Crep for site 1, then V0 = P2_0 * Crep(site1)
            st1 = stage_site(1)
            cr = cp.tile([128, Bs], F32, tag="crep")
            mm_pair(cr, e_sb[:], st1[:])
            csb = crep_sb.tile([128, Bs], F32, tag="csb")
            nc.scalar.copy(csb[:], cr[:])
            v = vpool.tile([128, Bs], F32R, tag="v")
            for c in range(n_chunks):
                nc.vector.tensor_mul(v[:, c * chunk:(c + 1) * chunk],
                                     p2[:, c * chunk:(c + 1) * chunk],
                                     csb[:, c * chunk:(c + 1) * chunk])

            # ---- main scan: step n applies mid core n-1 (site n), features site n+1
            for n in range(1, n_mid + 1):
                w_sb = wpool.tile([128, 128], F32R, tag="w")
                nc.sync.dma_start(w_sb[:], Ad.ap()[n - 1])
                p2 = pp.tile([128, Bs], F32, tag="p2")
                mm_pair(p2, w_sb[:], v[:])
                st = stage_site(n + 1)
                cr = cp.tile([128, Bs], F32, tag="crep")
                mm_pair(cr, e_sb[:], st[:])
                csb = crep_sb.tile([128, Bs], F32, tag="csb")
                nc.scalar.copy(csb[:], cr[:])
                v = vpool.tile([128, Bs], F32R, tag="v")
                for c in range(n_chunks):
                    nc.vector.tensor_mul(v[:, c * chunk:(c + 1) * chunk],
                                         p2[:, c * chunk:(c + 1) * chunk],
                                         csb[:, c * chunk:(c + 1) * chunk])
                last_p2 = p2

            # ---- endgame: logits + normalization
            # logits_raw [C, Bs] = WL^T @ V_last
            lg = pp.tile([C, Bs], F32, tag="p2")
            mm_pair(lg, wl_sb[:], v[:])
            # sumsq over state rows 0:64 of last P2 (pre-feature state = gamma*M)
            sq = endp.tile([D, Bs], F32R)
            nc.scalar.activation(sq[:], last_p2[0:D, :], AF.Square)
            ns = cp.tile([1, Bs], F32, tag="crep")
            mm_pair(ns, o64_sb[:], sq[:])
            rec = endp.tile([1, Bs], F32)
            nc.vector.reciprocal(rec[:], ns[:])          # 1/sumsq
            inv = endp.tile([1, Bs], F32R)
            nc.scalar.activation(inv[:], rec[:], AF.Sqrt)  # 1/||U||
            irep = cp.tile([C, Bs], F32, tag="crep")
            mm_pair(irep, o10_sb[:], inv[:])
            isb = endp.tile([C, Bs], F32)
            nc.scalar.copy(isb[:], irep[:])
            res = endp.tile([C, Bs], F32)
            nc.vector.tensor_mul(res[:], lg[:], isb[:])
            nc.sync.dma_start(out.ap(), res[:])

    nc.compile()
    return nc


def host_prep(x, core0, cores_mid, core_last, n_cores=N_CORES):
    """Shard + marshal inputs for the per-core program."""
    x = np.asarray(x, np.float32)
    core0 = np.asarray(core0, np.float32)
    cores_mid = np.asarray(cores_mid, np.float32)
    core_last = np.asarray(core_last, np.float32)
    B, n_sites = x.shape
    n_mid = n_sites - 2
    Bs = B // n_cores

    # doubled, alpha-scaled mid cores: Ad[n] = [[A0;A1] | [A0;A1]]  [128, 128]
    Aaug = (ALPHA * cores_mid).reshape(n_mid, 2 * D, D)       # [n, 128, 64]
    Ad = np.concatenate([Aaug, Aaug], axis=2)                  # [n, 128, 128]
    Ad = np.ascontiguousarray(Ad, np.float32)

    W0 = np.concatenate([core0[:, 0, :], core0[:, 0, :]], axis=1)  # [2, 128]
    W0 = np.ascontiguousarray(W0, np.float32)
    WL = np.ascontiguousarray(core_last.reshape(2 * D, C), np.float32)  # [128, 10]

    E = np.zeros((2, 128), np.float32)
    E[0, :D] = 1.0
    E[1, D:] = 1.0
    ones64 = np.ones((D, 1), np.float32)
    ones10 = np.ones((1, C), np.float32)

    in_maps = []
    for c in range(n_cores):
        xs = x[c * Bs:(c + 1) * Bs]                     # [Bs, n_sites]
        xTs = np.ascontiguousarray(xs.T)                # [n_sites, Bs]
        in_maps.append({
            "xT": xTs, "Ad": Ad, "W0": W0, "WL": WL,
            "Esel": E, "ones64": ones64, "ones10": ones10,
        })
    return in_maps, Bs


_CACHE = {}


def _get_nc(n_sites, Bs):
    key = (n_sites, Bs)
    if key not in _CACHE:
        _CACHE[key] = build_nc(n_sites, Bs)
    return _CACHE[key]


def run(x, core0, cores_mid, core_last, trace=False, **kw):
    B, n_sites = np.asarray(x).shape
    in_maps, Bs = host_prep(x, core0, cores_mid, core_last)
    nc = _get_nc(n_sites, Bs)
    res = run_bass_kernel_spmd(nc, in_maps, core_ids=list(range(N_CORES)), trace=trace, **kw)
    outs = [r["out"].T for r in res.results]            # [Bs, C] each
    logits = np.concatenate(outs, axis=0).astype(np.float32)
    return logits, res


def kernel(x, core0, cores_mid, core_last):
    logits, _ = run(x, core0, cores_mid, core_last)
    return logits
